# revision 13
# baseline (speedup 1.0000x reference)
"""TRN2 Bass kernel for nn_AttentionStoreProcessor (dense transformer attention).

Full (unsharded) inputs in, full output out. Internally:
  - CAPE rotation + softmax scale folded into Wq/Wk on host (exact linear algebra,
    per-frame 4x4 block-diagonal right-multiply).
  - Heads padded 20 -> 24 and tensor-parallel sharded 3 heads/core across 8 cores
    (zero weights for pad heads; their output contribution is exactly zero).
  - Per core: hs^T via PE transposes; fused QKV projections (float32r ~= tf32
    precision at full PE rate); scores^T = K^T(q)K(k) per (head, kt-tile);
    max-free softmax (scores are O(10), exp is safe in fp32) with sums obtained
    via a ones-column appended to V in the PV matmul; per-query normalization
    via a K=1 broadcast matmul; output projection from outT; residual/bias and
    the cross-core partial-sum reduction happen on host.
"""
import numpy as np
from contextlib import ExitStack

import concourse.bacc as bacc
import concourse.mybir as mybir
import concourse.tile as tile
from concourse.bass_utils import run_bass_kernel_spmd

F32 = mybir.dt.float32
F32R = mybir.dt.float32r
AF = mybir.ActivationFunctionType

HEADS = 20
PAD_HEADS = 24
HPC = 3  # heads per core
N_CORES = 8
S = 2048  # tokens
D = 1280  # channels
HD = 64  # head dim
L = 1024  # tokens per frame
KT = D // 128  # 10 contraction tiles for projections
TOKT = S // 128  # 16 token tiles

_CACHED_NC = None


def _build_nc():
    nc = bacc.Bacc("TRN2", debug=False, num_devices=N_CORES)

    hs = nc.dram_tensor("hs", [S, D], F32R, kind="ExternalInput").ap()
    # weight groups, host pre-laid-out as [t, 128, KT*cols] (ktile-major free dim)
    wg0 = nc.dram_tensor("wg0", [2, 128, KT * 128], F32R, kind="ExternalInput").ap()
    wg1 = nc.dram_tensor("wg1", [2, 128, KT * 128], F32R, kind="ExternalInput").ap()
    wg2 = nc.dram_tensor("wg2", [2, 128, KT * 128], F32R, kind="ExternalInput").ap()
    wv = nc.dram_tensor("wv", [128, KT * 192], F32R, kind="ExternalInput").ap()
    wo01 = nc.dram_tensor("wo01", [128, D], F32R, kind="ExternalInput").ap()
    wo2 = nc.dram_tensor("wo2", [64, D], F32R, kind="ExternalInput").ap()
    ident = nc.dram_tensor("ident", [128, 128], F32R, kind="ExternalInput").ap()
    ones = nc.dram_tensor("ones", [128, 64], F32R, kind="ExternalInput").ap()
    out = nc.dram_tensor("out", [S, D], F32, kind="ExternalOutput").ap()

    hs_r = hs.rearrange("(n p) d -> n p d", p=128)
    out_r = out.rearrange("(n p) d -> n p d", p=128)

    with (
        tile.TileContext(nc) as tc,
        ExitStack() as ctx,
        nc.allow_low_precision(reason="float32r (~tf32) used deliberately"),
    ):
        persist = ctx.enter_context(tc.tile_pool(name="persist", bufs=1))
        hsin_pool = tc.alloc_tile_pool(name="hsin", bufs=5)
        psT = tc.alloc_tile_pool(name="psT", bufs=3, space="PSUM")

        # ---- persistent SBUF tensors ----
        ones_sb = persist.tile([128, 64], F32R, tag="ones")
        nc.sync.dma_start(ones_sb[:], ones[:])

        s1 = tc.alloc_tile_pool(name="s1", bufs=1)
        ident_sb = s1.tile([128, 128], F32R, tag="ident")
        nc.sync.dma_start(ident_sb[:], ident[:])

        wg_sb = []
        for t in range(2):
            row = []
            for g, wsrc in enumerate((wg0, wg1, wg2)):
                w = s1.tile([128, KT * 128], F32R, tag=f"wg{t}{g}", name=f"wg{t}{g}")
                nc.sync.dma_start(w[:], wsrc[t])
                row.append(w)
            wg_sb.append(row)
        wv_sb = s1.tile([128, KT * 192], F32R, tag="wv")
        nc.sync.dma_start(wv_sb[:], wv[:])

        hsT = [s1.tile([128, S], F32R, tag=f"hsT{k}", name=f"hsT{k}") for k in range(KT)]
        QA = persist.tile([128, S], F32R, tag="QA")  # rows 0:64 qT_h0, 64:128 qT_h1
        KA = persist.tile([128, S], F32R, tag="KA")  # rows 0:64 kT_h0, 64:128 kT_h1
        QK2 = persist.tile([128, S], F32R, tag="QK2")  # rows 0:64 q2, 64:128 k2
        QB2 = persist.tile([128, S], F32R, tag="QB2")  # rows 64:128 <- q2 (shifted)
        v195 = persist.tile([128, TOKT, 195], F32R, tag="v195")

        # ones columns of v_ext (col 65h+64 = 1.0)
        for h in range(HPC):
            nc.vector.tensor_copy(v195[:, :, 65 * h + 64], ones_sb[:, 0:TOKT])

        # ---- stage T: load hs tiles and transpose into hsT ----
        hs_sb = []
        for n in range(TOKT):
            t_in = hsin_pool.tile([128, D], F32R, tag="hsin", name=f"hsin{n}")
            nc.sync.dma_start(t_in[:], hs_r[n])
            hs_sb.append(t_in)
        for grp in range(4):  # groups of 4 token tiles
            for k in range(KT):
                tp = psT.tile([128, 512], F32R, tag="ps512", name=f"tp{grp}_{k}")
                for j in range(4):
                    n = grp * 4 + j
                    nc.tensor.transpose(
                        tp[:, j * 128 : (j + 1) * 128],
                        hs_sb[n][:, k * 128 : (k + 1) * 128],
                        ident_sb[:],
                    )
                nc.vector.tensor_copy(
                    hsT[k][:, grp * 512 : (grp + 1) * 512], tp[:]
                )

        # ---- stage P: projections ----
        # q/k groups: per 512-token chunk (4 chunks; chunk//2 selects CAPE frame t)
        for ch in range(4):
            t = ch // 2
            qs = slice(ch * 512, (ch + 1) * 512)
            for g, dest in enumerate((QA, KA, QK2)):
                pp = psT.tile([128, 512], F32, tag="ps512", name=f"pp{ch}_{g}")
                for k in range(KT):
                    nc.tensor.matmul(
                        pp[:],
                        wg_sb[t][g][:, k * 128 : (k + 1) * 128],
                        hsT[k][:, qs],
                        start=(k == 0),
                        stop=(k == KT - 1),
                    )
                nc.vector.tensor_copy(dest[:, qs], pp[:])
            # v for the 4 token tiles of this chunk
            for j in range(4):
                n = ch * 4 + j
                vp = psT.tile([128, 192], F32, tag="ps512", name=f"vp{n}")
                for k in range(KT):
                    nc.tensor.matmul(
                        vp[:],
                        hsT[k][:, n * 128 : (n + 1) * 128],
                        wv_sb[:, k * 192 : (k + 1) * 192],
                        start=(k == 0),
                        stop=(k == KT - 1),
                    )
                for h in range(HPC):
                    nc.vector.tensor_copy(
                        v195[:, n, 65 * h : 65 * h + 64],
                        vp[:, h * 64 : (h + 1) * 64],
                    )

        # shift q2 (QK2 rows 0:64) up to rows 64:128 so h2 scores run at base 64
        nc.sync.dma_start(QB2[64:128, :], QK2[0:64, :])

        # free stage-1 SBUF (hsT, projection weights, hs input staging)
        s1.release()
        psT.release()
        hsin_pool.release()

        # late-stage tensors (created after hsT frees up SBUF)
        persistB = ctx.enter_context(tc.tile_pool(name="persistB", bufs=1))
        u_pool = tc.alloc_tile_pool(name="u", bufs=3)
        rc_pool = tc.alloc_tile_pool(name="rc", bufs=2)
        outT01 = persistB.tile([128, S], F32R, tag="outT01")
        outT2 = persistB.tile([64, S], F32R, tag="outT2")
        oT1tmp = persistB.tile([64, S], F32R, tag="oT1tmp")
        wo01_sb = persistB.tile([128, D], F32R, tag="wo01")
        nc.sync.dma_start(wo01_sb[:], wo01[:])
        wo2_sb = persistB.tile([64, D], F32R, tag="wo2")
        nc.sync.dma_start(wo2_sb[:], wo2[:])

        sc_pool = tc.alloc_tile_pool(name="sc", bufs=2, space="PSUM")
        pv_pool = tc.alloc_tile_pool(name="pv", bufs=4, space="PSUM")

        # ---- attention ----
        # head operand map: (lhsT=kT slice source+rows, rhs=qT source+rows)
        def head_ops(h):
            if h == 0:
                return KA, slice(0, 64), QA, slice(0, 64)
            if h == 1:
                return KA, slice(64, 128), QA, slice(64, 128)
            return QK2, slice(64, 128), QB2, slice(64, 128)

        def attend(h, qh, pv_tiles):
            ksrc, krows, qsrc, qrows = head_ops(h)
            for kt in range(TOKT):
                sc = sc_pool.tile([128, 1024], F32, tag="sc", name=f"sc2_{qh}_{kt}")
                for half in range(2):
                    nc.tensor.matmul(
                        sc[:, half * 512 : (half + 1) * 512],
                        ksrc[krows, kt * 128 : (kt + 1) * 128],
                        qsrc[qrows, qh * 1024 + half * 512 : qh * 1024 + (half + 1) * 512],
                        start=True,
                        stop=True,
                    )
                u = u_pool.tile([128, 1024], F32R, tag="u", name=f"u2_{qh}_{kt}")
                nc.scalar.activation(u[:], sc[:], AF.Exp)
                for sub in range(2):
                    nc.tensor.matmul(
                        pv_tiles[sub],
                        v195[:, kt, 65 * h : 65 * h + 65],
                        u[:, sub * 512 : (sub + 1) * 512],
                        start=(kt == 0),
                        stop=(kt == TOKT - 1),
                    )

        def normalize(h, qh, pv_tiles):
            # pv tile: rows 0:64 = unnormalized outT, row 64 = sums
            for sub in range(2):
                pvt = pv_tiles[sub]
                qcol = slice(qh * 1024 + sub * 512, qh * 1024 + (sub + 1) * 512)
                rc = rc_pool.tile([65, 512], F32R, tag="rc", name=f"rc{h}_{qh}_{sub}")
                nc.vector.reciprocal(rc[64:65, :], pvt[64:65, :])
                bc = sc_pool.tile([64, 512], F32, tag="sc", name=f"bc{h}_{qh}_{sub}")
                nc.tensor.matmul(
                    bc[:], ones_sb[64:65, :], rc[64:65, :], start=True, stop=True
                )
                bcs = rc_pool.tile([64, 512], F32, tag="bcs", name=f"bcs{h}_{qh}_{sub}")
                nc.vector.tensor_copy(bcs[:], bc[:])
                if h == 0:
                    dest = outT01[0:64, qcol]
                elif h == 1:
                    dest = oT1tmp[:, qcol]
                else:
                    dest = outT2[:, qcol]
                nc.vector.tensor_mul(dest, pvt[0:64, :], bcs[:])

        for qh in range(2):
            # paired heads 0,1 (row groups 0 / 64 run concurrently on PE)
            pv01 = {
                h: [
                    pv_pool.tile([65, 512], F32, tag="pv", name=f"pv{qh}_{h}_{s_}")
                    for s_ in range(2)
                ]
                for h in range(2)
            }
            # interleave emission per kt so PE packs h0/h1 score matmuls
            for kt in range(TOKT):
                for h in range(2):
                    ksrc, krows, qsrc, qrows = head_ops(h)
                    sc = sc_pool.tile([128, 1024], F32, tag="sc", name=f"sc{qh}_{kt}_{h}")
                    for half in range(2):
                        nc.tensor.matmul(
                            sc[:, half * 512 : (half + 1) * 512],
                            ksrc[krows, kt * 128 : (kt + 1) * 128],
                            qsrc[
                                qrows,
                                qh * 1024 + half * 512 : qh * 1024 + (half + 1) * 512,
                            ],
                            start=True,
                            stop=True,
                        )
                    u = u_pool.tile([128, 1024], F32R, tag="u", name=f"u{qh}_{kt}_{h}")
                    nc.scalar.activation(u[:], sc[:], AF.Exp)
                    for sub in range(2):
                        nc.tensor.matmul(
                            pv01[h][sub],
                            v195[:, kt, 65 * h : 65 * h + 65],
                            u[:, sub * 512 : (sub + 1) * 512],
                            start=(kt == 0),
                            stop=(kt == TOKT - 1),
                        )
            for h in range(2):
                normalize(h, qh, pv01[h])
            # head 2 alone
            pv2 = [pv_pool.tile([65, 512], F32, tag="pv", name=f"pv2_{qh}_{s_}") for s_ in range(2)]
            attend(2, qh, pv2)
            normalize(2, qh, pv2)

        # move h1's outT into rows 64:128 of outT01 (partition shift via DMA)
        nc.sync.dma_start(outT01[64:128, :], oT1tmp[:, :])

        pv_pool.release()
        sc_pool.release()
        rc_pool.release()
        u_pool.release()
        opP = tc.alloc_tile_pool(name="opP", bufs=3, space="PSUM")
        osb_pool = tc.alloc_tile_pool(name="osb", bufs=3)

        # ---- output projection ----
        for n in range(TOKT):
            ts = slice(n * 128, (n + 1) * 128)
            for dc, (off, w) in enumerate(((0, 512), (512, 512), (1024, 256))):
                op = opP.tile([128, 512], F32, tag="op", name=f"op{n}_{dc}")
                nc.tensor.matmul(
                    op[:, 0:w],
                    outT01[:, ts],
                    wo01_sb[:, off : off + w],
                    start=True,
                    stop=False,
                )
                nc.tensor.matmul(
                    op[:, 0:w],
                    outT2[:, ts],
                    wo2_sb[:, off : off + w],
                    start=False,
                    stop=True,
                )
                ob = osb_pool.tile([128, 512], F32, tag="osb", name=f"ob{n}_{dc}")
                nc.vector.tensor_copy(ob[:, 0:w], op[:, 0:w])
                nc.sync.dma_start(out_r[n][:, off : off + w], ob[:, 0:w])

        osb_pool.release()
        opP.release()

    nc.compile()
    return nc


def _get_nc():
    global _CACHED_NC
    if _CACHED_NC is None:
        _CACHED_NC = _build_nc()
    return _CACHED_NC


def _fold_cape(W, P):
    """W @ blockdiag(P) for 4x4 P repeated along channels: exact CAPE fold."""
    d = W.shape[1]
    W4 = W.reshape(W.shape[0], d // 4, 4)
    return np.einsum("cik,kj->cij", W4, P, optimize=True).reshape(W.shape[0], d)


def _prep_in_maps(hidden_states, p_out, p_out_inv, Wq, Wk, Wv, Wo):
    scale = HD ** -0.5
    hs2 = np.ascontiguousarray(
        hidden_states.reshape(S, D), dtype=np.float32
    )

    FEAT = PAD_HEADS * HD  # 1536
    Wq_eff = np.zeros((2, D, FEAT), np.float32)
    Wk_eff = np.zeros((2, D, FEAT), np.float32)
    for t in range(2):
        Wq_eff[t, :, :D] = _fold_cape(Wq, p_out_inv[0, t]) * scale
        Wk_eff[t, :, :D] = _fold_cape(Wk, p_out[0, t])
    Wv_pad = np.zeros((D, FEAT), np.float32)
    Wv_pad[:, :D] = Wv
    Wo_pad = np.zeros((FEAT, D), np.float32)
    Wo_pad[:D, :] = Wo

    def klayout(W, cols):
        # [1280, cols] -> [128, KT*cols] with ktile-major free dim
        return np.ascontiguousarray(
            W.reshape(KT, 128, cols).transpose(1, 0, 2).reshape(128, KT * cols)
        )

    ident = np.eye(128, dtype=np.float32)
    in_maps = []
    for c in range(N_CORES):
        A = c * HPC * HD
        wg0 = np.stack(
            [klayout(Wq_eff[t][:, A : A + 128], 128) for t in range(2)]
        )
        wg1 = np.stack(
            [klayout(Wk_eff[t][:, A : A + 128], 128) for t in range(2)]
        )
        wg2 = np.stack(
            [
                klayout(
                    np.concatenate(
                        [
                            Wq_eff[t][:, A + 128 : A + 192],
                            Wk_eff[t][:, A + 128 : A + 192],
                        ],
                        axis=1,
                    ),
                    128,
                )
                for t in range(2)
            ]
        )
        wv_l = klayout(Wv_pad[:, A : A + 192], 192)
        wo01 = np.ascontiguousarray(Wo_pad[A : A + 128, :])
        wo2 = np.ascontiguousarray(Wo_pad[A + 128 : A + 192, :])
        in_maps.append(
            {
                "hs": hs2,
                "wg0": wg0,
                "wg1": wg1,
                "wg2": wg2,
                "wv": wv_l,
                "wo01": wo01,
                "wo2": wo2,
                "ident": ident,
                "ones": np.ones((128, 64), np.float32),
            }
        )
    return in_maps


def kernel(hidden_states, p_out, p_out_inv, Wq, Wk, Wv, Wo, bo):
    hidden_states = np.asarray(hidden_states, dtype=np.float32)
    in_maps = _prep_in_maps(
        hidden_states,
        np.asarray(p_out, np.float32),
        np.asarray(p_out_inv, np.float32),
        np.asarray(Wq, np.float32),
        np.asarray(Wk, np.float32),
        np.asarray(Wv, np.float32),
        np.asarray(Wo, np.float32),
    )
    nc = _get_nc()
    res = run_bass_kernel_spmd(nc, in_maps, core_ids=list(range(N_CORES)))
    acc = np.zeros((S, D), np.float32)
    for c in range(N_CORES):
        acc += res.results[c]["out"]
    acc += np.asarray(bo, np.float32)[None, :]
    out = acc.reshape(2, L, D) + hidden_states
    return out


# revision 14
# speedup vs baseline: 89055.2772x; 89055.2772x over previous
"""TRN2 Bass kernel for nn_AttentionStoreProcessor (dense transformer attention).

Full (unsharded) inputs in, full output out. Strategy:
  - CAPE rotation + softmax scale folded into Wq/Wk on host (exact linear
    algebra: per-frame 4x4 block-diagonal right-multiply of the weights).
  - Heads padded 20 -> 24, tensor-parallel 3 heads/core across 8 cores
    (zero weights for pad heads; their output contribution is exactly zero).
  - Per core: hs^T via PE transposes; fused QKV projections in float32r
    (~tf32 precision at full PE rate); scores^T per (head, kt-tile); max-free
    softmax (scores are O(10) so exp never overflows in fp32); row sums from a
    ones-column appended to V inside the PV matmul; per-query normalization via
    a K=1 broadcast matmul; output projection from outT. Residual + bias + the
    cross-core partial-sum reduction happen on host (tiny).
"""
import numpy as np
from contextlib import ExitStack

import concourse.bacc as bacc
import concourse.mybir as mybir
import concourse.tile as tile
from concourse.bass_utils import run_bass_kernel_spmd

F32 = mybir.dt.float32
F32R = mybir.dt.float32r
AF = mybir.ActivationFunctionType

HEADS = 20
PAD_HEADS = 24
HPC = 3  # heads per core
N_CORES = 8
S = 2048  # tokens
D = 1280  # channels
HD = 64  # head dim
L = 1024  # tokens per frame
KT = D // 128  # 10 contraction tiles for projections
TOKT = S // 128  # 16 token tiles

_CACHED_NC = {}


def _emit_body(nc, tc, aps, r):
    """Emit one full forward pass. r = repetition index (names/tags suffix)."""
    hs_r, wgs, wv, wo01, wo2, ident, ones, out_r = aps

    with ExitStack() as ctx:
        persist = ctx.enter_context(tc.tile_pool(name=f"persist{r}", bufs=1))
        psT = tc.alloc_tile_pool(name=f"psT{r}", bufs=3, space="PSUM")
        hsin_pool = tc.alloc_tile_pool(name=f"hsin{r}", bufs=5)
        s1 = tc.alloc_tile_pool(name=f"s1{r}", bufs=1)

        ones_sb = persist.tile([128, 64], F32R, tag="ones", name=f"ones{r}")
        nc.sync.dma_start(ones_sb[:], ones[:])
        ident_sb = s1.tile([128, 128], F32R, tag="ident", name=f"ident{r}")
        nc.sync.dma_start(ident_sb[:], ident[:])

        wg_sb = []
        for t in range(2):
            row = []
            for g in range(3):
                w = s1.tile(
                    [128, KT * 128], F32R, tag=f"wg{t}{g}", name=f"wg{r}_{t}{g}"
                )
                nc.sync.dma_start(w[:], wgs[g][t])
                row.append(w)
            wg_sb.append(row)
        wv_sb = s1.tile([128, KT * 192], F32R, tag="wv", name=f"wv{r}")
        nc.sync.dma_start(wv_sb[:], wv[:])

        hsT = [
            s1.tile([128, S], F32R, tag=f"hsT{k}", name=f"hsT{r}_{k}")
            for k in range(KT)
        ]
        QA = persist.tile([128, S], F32R, tag="QA", name=f"QA{r}")
        KA = persist.tile([128, S], F32R, tag="KA", name=f"KA{r}")
        QK2 = persist.tile([128, S], F32R, tag="QK2", name=f"QK2{r}")
        QB2 = persist.tile([128, S], F32R, tag="QB2", name=f"QB2{r}")
        v195 = persist.tile([128, TOKT, 195], F32R, tag="v195", name=f"v195{r}")

        for h in range(HPC):
            nc.vector.tensor_copy(v195[:, :, 65 * h + 64], ones_sb[:, 0:TOKT])

        # ---- stage T: load hs tiles, PE-transpose into hsT ----
        hs_sb = []
        for n in range(TOKT):
            t_in = hsin_pool.tile([128, D], F32R, tag="hsin", name=f"hsin{r}_{n}")
            nc.sync.dma_start(t_in[:], hs_r[n])
            hs_sb.append(t_in)
        for grp in range(4):
            for k in range(KT):
                tp = psT.tile([128, 512], F32R, tag="ps512", name=f"tp{r}_{grp}_{k}")
                for j in range(4):
                    n = grp * 4 + j
                    nc.tensor.transpose(
                        tp[:, j * 128 : (j + 1) * 128],
                        hs_sb[n][:, k * 128 : (k + 1) * 128],
                        ident_sb[:],
                    )
                nc.vector.tensor_copy(hsT[k][:, grp * 512 : (grp + 1) * 512], tp[:])

        # ---- stage P: projections ----
        for ch in range(4):
            t = ch // 2  # CAPE frame
            qs = slice(ch * 512, (ch + 1) * 512)
            for g, dest in enumerate((QA, KA, QK2)):
                pp = psT.tile([128, 512], F32, tag="ps512", name=f"pp{r}_{ch}_{g}")
                for k in range(KT):
                    nc.tensor.matmul(
                        pp[:],
                        wg_sb[t][g][:, k * 128 : (k + 1) * 128],
                        hsT[k][:, qs],
                        start=(k == 0),
                        stop=(k == KT - 1),
                    )
                nc.vector.tensor_copy(dest[:, qs], pp[:])
            for j in range(4):
                n = ch * 4 + j
                vp = psT.tile([128, 192], F32, tag="ps512", name=f"vp{r}_{n}")
                for k in range(KT):
                    nc.tensor.matmul(
                        vp[:],
                        hsT[k][:, n * 128 : (n + 1) * 128],
                        wv_sb[:, k * 192 : (k + 1) * 192],
                        start=(k == 0),
                        stop=(k == KT - 1),
                    )
                for h in range(HPC):
                    nc.vector.tensor_copy(
                        v195[:, n, 65 * h : 65 * h + 64],
                        vp[:, h * 64 : (h + 1) * 64],
                    )

        # shift q2 (QK2 rows 0:64) to rows 64:128 so h2 scores run at base 64
        nc.sync.dma_start(QB2[64:128, :], QK2[0:64, :])

        s1.release()
        psT.release()
        hsin_pool.release()

        # ---- attention-phase tensors/pools ----
        persistB = ctx.enter_context(tc.tile_pool(name=f"persistB{r}", bufs=1))
        u_pool = tc.alloc_tile_pool(name=f"u{r}", bufs=3)
        rc_pool = tc.alloc_tile_pool(name=f"rc{r}", bufs=2)
        outT01 = persistB.tile([128, S], F32R, tag="outT01", name=f"outT01{r}")
        outT2 = persistB.tile([64, S], F32R, tag="outT2", name=f"outT2{r}")
        oT1tmp = persistB.tile([64, S], F32R, tag="oT1tmp", name=f"oT1tmp{r}")
        wo01_sb = persistB.tile([128, D], F32R, tag="wo01", name=f"wo01{r}")
        nc.sync.dma_start(wo01_sb[:], wo01[:])
        wo2_sb = persistB.tile([64, D], F32R, tag="wo2", name=f"wo2{r}")
        nc.sync.dma_start(wo2_sb[:], wo2[:])

        sc_pool = tc.alloc_tile_pool(name=f"sc{r}", bufs=2, space="PSUM")
        pv_pool = tc.alloc_tile_pool(name=f"pv{r}", bufs=4, space="PSUM")

        def head_ops(h):
            # (kT source, rows, qT source, rows) — both at the same base
            if h == 0:
                return KA, slice(0, 64), QA, slice(0, 64)
            if h == 1:
                return KA, slice(64, 128), QA, slice(64, 128)
            return QK2, slice(64, 128), QB2, slice(64, 128)

        def score_pv(h, qh, kt, pv_tiles, name):
            ksrc, krows, qsrc, qrows = head_ops(h)
            sc = sc_pool.tile([128, 1024], F32, tag="sc", name=f"sc{name}")
            for half in range(2):
                nc.tensor.matmul(
                    sc[:, half * 512 : (half + 1) * 512],
                    ksrc[krows, kt * 128 : (kt + 1) * 128],
                    qsrc[
                        qrows,
                        qh * 1024 + half * 512 : qh * 1024 + (half + 1) * 512,
                    ],
                    start=True,
                    stop=True,
                )
            u = u_pool.tile([128, 1024], F32R, tag="u", name=f"u{name}")
            nc.scalar.activation(u[:], sc[:], AF.Exp)
            for sub in range(2):
                nc.tensor.matmul(
                    pv_tiles[sub],
                    v195[:, kt, 65 * h : 65 * h + 65],
                    u[:, sub * 512 : (sub + 1) * 512],
                    start=(kt == 0),
                    stop=(kt == TOKT - 1),
                )

        def normalize(h, qh, pv_tiles):
            for sub in range(2):
                pvt = pv_tiles[sub]
                qcol = slice(qh * 1024 + sub * 512, qh * 1024 + (sub + 1) * 512)
                nm = f"{r}_{h}_{qh}_{sub}"
                rc = rc_pool.tile([65, 512], F32R, tag="rc", name=f"rc{nm}")
                nc.vector.reciprocal(rc[64:65, :], pvt[64:65, :])
                bc = sc_pool.tile([64, 512], F32, tag="sc", name=f"bc{nm}")
                nc.tensor.matmul(
                    bc[:], ones_sb[64:65, :], rc[64:65, :], start=True, stop=True
                )
                bcs = rc_pool.tile([64, 512], F32, tag="bcs", name=f"bcs{nm}")
                nc.vector.tensor_copy(bcs[:], bc[:])
                if h == 0:
                    dest = outT01[0:64, qcol]
                elif h == 1:
                    dest = oT1tmp[:, qcol]
                else:
                    dest = outT2[:, qcol]
                nc.vector.tensor_mul(dest, pvt[0:64, :], bcs[:])

        for qh in range(2):
            # heads 0,1 interleaved: their score matmuls sit in different PE
            # row groups (base 0 / base 64) and run concurrently
            pv01 = {
                h: [
                    pv_pool.tile(
                        [65, 512], F32, tag="pv", name=f"pv{r}_{qh}_{h}_{s_}"
                    )
                    for s_ in range(2)
                ]
                for h in range(2)
            }
            for kt in range(TOKT):
                for h in range(2):
                    score_pv(h, qh, kt, pv01[h], f"{r}_{qh}_{kt}_{h}")
            for h in range(2):
                normalize(h, qh, pv01[h])
            pv2 = [
                pv_pool.tile([65, 512], F32, tag="pv", name=f"pv2_{r}_{qh}_{s_}")
                for s_ in range(2)
            ]
            for kt in range(TOKT):
                score_pv(2, qh, kt, pv2, f"{r}_{qh}_{kt}_2")
            normalize(2, qh, pv2)

        # h1's outT into rows 64:128 of outT01 (partition shift via DMA)
        nc.sync.dma_start(outT01[64:128, :], oT1tmp[:, :])

        pv_pool.release()
        sc_pool.release()
        rc_pool.release()
        u_pool.release()
        opP = tc.alloc_tile_pool(name=f"opP{r}", bufs=3, space="PSUM")
        osb_pool = tc.alloc_tile_pool(name=f"osb{r}", bufs=3)

        # ---- output projection ----
        for n in range(TOKT):
            ts = slice(n * 128, (n + 1) * 128)
            for dc, (off, w) in enumerate(((0, 512), (512, 512), (1024, 256))):
                op = opP.tile([128, 512], F32, tag="op", name=f"op{r}_{n}_{dc}")
                nc.tensor.matmul(
                    op[:, 0:w],
                    outT01[:, ts],
                    wo01_sb[:, off : off + w],
                    start=True,
                    stop=False,
                )
                nc.tensor.matmul(
                    op[:, 0:w],
                    outT2[:, ts],
                    wo2_sb[:, off : off + w],
                    start=False,
                    stop=True,
                )
                ob = osb_pool.tile(
                    [128, 512], F32, tag="osb", name=f"ob{r}_{n}_{dc}"
                )
                nc.vector.tensor_copy(ob[:, 0:w], op[:, 0:w])
                nc.sync.dma_start(out_r[n][:, off : off + w], ob[:, 0:w])

        osb_pool.release()
        opP.release()


def _build_nc(repeat=1):
    nc = bacc.Bacc("TRN2", debug=False, num_devices=N_CORES)

    hs = nc.dram_tensor("hs", [S, D], F32R, kind="ExternalInput").ap()
    wg0 = nc.dram_tensor("wg0", [2, 128, KT * 128], F32R, kind="ExternalInput").ap()
    wg1 = nc.dram_tensor("wg1", [2, 128, KT * 128], F32R, kind="ExternalInput").ap()
    wg2 = nc.dram_tensor("wg2", [2, 128, KT * 128], F32R, kind="ExternalInput").ap()
    wv = nc.dram_tensor("wv", [128, KT * 192], F32R, kind="ExternalInput").ap()
    wo01 = nc.dram_tensor("wo01", [128, D], F32R, kind="ExternalInput").ap()
    wo2 = nc.dram_tensor("wo2", [64, D], F32R, kind="ExternalInput").ap()
    ident = nc.dram_tensor("ident", [128, 128], F32R, kind="ExternalInput").ap()
    ones = nc.dram_tensor("ones", [128, 64], F32R, kind="ExternalInput").ap()
    out = nc.dram_tensor("out", [S, D], F32, kind="ExternalOutput").ap()

    hs_r = hs.rearrange("(n p) d -> n p d", p=128)
    out_r = out.rearrange("(n p) d -> n p d", p=128)
    aps = (hs_r, (wg0, wg1, wg2), wv, wo01, wo2, ident, ones, out_r)

    with (
        tile.TileContext(nc) as tc,
        nc.allow_low_precision(reason="float32r (~tf32) used deliberately"),
    ):
        for rep in range(repeat):
            _emit_body(nc, tc, aps, rep)

    nc.compile()
    return nc


def _get_nc(repeat=1):
    if repeat not in _CACHED_NC:
        _CACHED_NC[repeat] = _build_nc(repeat)
    return _CACHED_NC[repeat]


def _fold_cape(W, P):
    """W @ blockdiag(P) for a 4x4 P repeated along channels (exact CAPE fold)."""
    d = W.shape[1]
    W4 = W.reshape(W.shape[0], d // 4, 4)
    return np.einsum("cik,kj->cij", W4, P, optimize=True).reshape(W.shape[0], d)


def _prep_in_maps(hidden_states, p_out, p_out_inv, Wq, Wk, Wv, Wo):
    scale = HD ** -0.5
    hs2 = np.ascontiguousarray(hidden_states.reshape(S, D), dtype=np.float32)

    FEAT = PAD_HEADS * HD  # 1536
    Wq_eff = np.zeros((2, D, FEAT), np.float32)
    Wk_eff = np.zeros((2, D, FEAT), np.float32)
    for t in range(2):
        Wq_eff[t, :, :D] = _fold_cape(Wq, p_out_inv[0, t]) * scale
        Wk_eff[t, :, :D] = _fold_cape(Wk, p_out[0, t])
    Wv_pad = np.zeros((D, FEAT), np.float32)
    Wv_pad[:, :D] = Wv
    Wo_pad = np.zeros((FEAT, D), np.float32)
    Wo_pad[:D, :] = Wo

    def klayout(W, cols):
        # [1280, cols] -> [128, KT*cols], ktile-major along the free dim
        return np.ascontiguousarray(
            W.reshape(KT, 128, cols).transpose(1, 0, 2).reshape(128, KT * cols)
        )

    ident = np.eye(128, dtype=np.float32)
    ones = np.ones((128, 64), np.float32)
    in_maps = []
    for c in range(N_CORES):
        A = c * HPC * HD
        wg0 = np.stack([klayout(Wq_eff[t][:, A : A + 128], 128) for t in range(2)])
        wg1 = np.stack([klayout(Wk_eff[t][:, A : A + 128], 128) for t in range(2)])
        wg2 = np.stack(
            [
                klayout(
                    np.concatenate(
                        [
                            Wq_eff[t][:, A + 128 : A + 192],
                            Wk_eff[t][:, A + 128 : A + 192],
                        ],
                        axis=1,
                    ),
                    128,
                )
                for t in range(2)
            ]
        )
        in_maps.append(
            {
                "hs": hs2,
                "wg0": wg0,
                "wg1": wg1,
                "wg2": wg2,
                "wv": klayout(Wv_pad[:, A : A + 192], 192),
                "wo01": np.ascontiguousarray(Wo_pad[A : A + 128, :]),
                "wo2": np.ascontiguousarray(Wo_pad[A + 128 : A + 192, :]),
                "ident": ident,
                "ones": ones,
            }
        )
    return in_maps


def kernel(hidden_states, p_out, p_out_inv, Wq, Wk, Wv, Wo, bo, _repeat=1):
    hidden_states = np.asarray(hidden_states, dtype=np.float32)
    in_maps = _prep_in_maps(
        hidden_states,
        np.asarray(p_out, np.float32),
        np.asarray(p_out_inv, np.float32),
        np.asarray(Wq, np.float32),
        np.asarray(Wk, np.float32),
        np.asarray(Wv, np.float32),
        np.asarray(Wo, np.float32),
    )
    nc = _get_nc(_repeat)
    res = run_bass_kernel_spmd(nc, in_maps, core_ids=list(range(N_CORES)))
    acc = np.zeros((S, D), np.float32)
    for c in range(N_CORES):
        acc += res.results[c]["out"]
    acc += np.asarray(bo, np.float32)[None, :]
    out = acc.reshape(2, L, D) + hidden_states
    return out
